# revision 1
# baseline (speedup 1.0000x reference)
"""Trainium2 Bass kernel for nn_BiquadCoeffFilter_31628139167986.

Reference computation (per batch row, T = 262144 samples):
  logits = linear-interp of 256 control points -> T samples (5 channels)
  a1 = 2*tanh(l0)*stab ; a2 = 0.5*((2-|a1|)*tanh(l1)*stab + |a1|)  (stab = 1-1e-3)
  IIR:  y[t] = x[t] - a1[t]*y[t-1] - a2[t]*y[t-2]
  FIR:  out[t] = b0[t]*y[t] + b1[t]*y[t-1] + b2[t]*y[t-2],  b = logits[..., 2:5]

Sharding: pure data parallel, 4 batch rows per core x 8 NeuronCores (SPMD).

Per-core pipeline:
  A. Coefficient generation in SEGMENT layout (partition = one interpolation
     segment window of 1032 samples; the interpolated logit is affine in the
     in-window position, so ScalarE computes tanh(w*d + v0) with per-partition
     scale/bias, fusing interpolation into the activation).  na1 = -a1 and
     na2 = -a2 are scattered to a time-linear DRAM stage.
  B. Chunked 3-solution scan in SCAN layout [128 partitions = 8192-sample
     stretches, 256 chunks x 32 steps]: per chunk the zero-state response
     y_zero (DVE) and the two homogeneous responses h1 (DVE) / h2 (GPSIMD)
     are computed in 32 vectorized steps.
  C. Per-chunk affine transfer maps (2x2 M, offset p) are prefix-composed
     hierarchically (chunks-in-block, blocks-in-stretch on-chip; stretch
     chain per row via a small DRAM transpose) giving the exact state
     entering every chunk; correction y = y_zero + alpha*h1 + beta*h2 with
     stride-0 broadcast of the per-chunk states.
  D. FIR in segment layout (b coefficients generated in place), scattered
     straight to the output.
"""
import sys
sys.path.insert(0, '/opt/trn_rl_repo')
import numpy as np

B, T = 32, 262144
NSEG = 255
SEGLEN = 87381      # (T-1)/3 ; 3 super-blocks x 85 segments per row
SUP = 85
ROWS = 4
NCORES = 8
L1 = 32             # chunk length
NSTR = 32           # stretches per row
STR = T // NSTR     # 8192
CPS = STR // L1     # 256 chunks per stretch
WIN = 1032
PAD = 4
DELTA = float(NSEG) / float(T - 1)
STAB = 1.0 - 1e-3

_PATCHED = False


def _patch_tile_drain():
    """This toolchain allows a single sem wait per instruction; split the tile
    tail-drain's accumulated waits across chained drain instructions."""
    global _PATCHED
    if _PATCHED:
        return
    from concourse import tile, mybir
    from concourse.vector_clock import ScopedClock

    def _drain_and_barrier_split(self, tick_clock, wait_clock):
        drain_inst = self.nc.sync.drain()
        wait_clock.add_sem_waits(
            drain_inst.ins, ScopedClock({None: tick_clock.global_clock}))
        si = drain_inst.ins.sync_info
        waits = list(si.on_wait or []) if si else []
        if len(waits) > 1:
            si.on_wait = waits[:1]
            for i in range(1, len(waits)):
                d2 = self.nc.sync.drain()
                d2.ins.sync_info = mybir.SyncInfo(on_wait=[waits[i]], on_update=[])
        self.nc.all_engine_barrier()
        assert self.sems is not None
        popped = self.nc._tile_sem_poison_stack.pop()
        assert popped is self._sem_poison
        self.nc.clear_and_free_semaphores(list(self.sems.allocated().values()))
        self.nc.all_engine_barrier()

    tile.TileContext._drain_and_barrier = _drain_and_barrier_split
    _PATCHED = True


def _fix_multi_waits(nc):
    """Hoist extra sem waits onto same-engine nops (1-wait codegen limit)."""
    from concourse import mybir

    def make_nop(engine):
        bi = nc.engines[engine].nop(nofuse=True, hint="wait_split")
        inst = bi.ins
        for f in nc.m.functions:
            for bb in f.blocks:
                il = bb.instructions
                if il and il[-1] is inst:
                    bb.instructions = il[:-1]
                    return inst
        raise RuntimeError("nop not found")

    for f in nc.m.functions:
        for bb in f.blocks:
            il = list(bb.instructions)
            out = []
            changed = False
            for inst in il:
                si = getattr(inst, 'sync_info', None)
                waits = list(si.on_wait or []) if si else []
                if len(waits) > 1 and getattr(inst, 'engine', None) is not None:
                    changed = True
                    extra, keep = waits[:-1], waits[-1:]
                    for w in extra:
                        nop = make_nop(inst.engine)
                        nop.sync_info = mybir.SyncInfo(on_wait=[w], on_update=[])
                        out.append(nop)
                    si.on_wait = keep
                out.append(inst)
            if changed:
                bb.instructions = out


def _lane_runs():
    """lane = r*255 + 85*k + sp  (row r, super-block k, segment sp).
    Runs of consecutive sp split at 128-partition tile boundaries.
    Returns (tile, part0, r, k, sp0, n)."""
    runs = []
    for r in range(ROWS):
        for k in range(3):
            base = r * NSEG + SUP * k
            sp = 0
            while sp < SUP:
                lane = base + sp
                tile_i, part = divmod(lane, 128)
                n = min(SUP - sp, 128 - part)
                runs.append((tile_i, part, r, k, sp, n))
                sp += n
    return runs


RUNS = _lane_runs()


def host_tables():
    w0 = np.zeros((128, 8, 1), np.float32)
    for r in range(ROWS):
        for k in range(3):
            for sp in range(SUP):
                lane = r * NSEG + SUP * k + sp
                seg = SUP * k + sp
                wstart = SEGLEN * k + 1028 * sp - 2
                w0[lane % 128, lane // 128, 0] = np.float64(wstart) * DELTA - seg
    iota = np.arange(WIN, dtype=np.float32)[None, :].repeat(128, 0)
    return w0, iota


def host_v0v1(cl_rows):
    """Per-lane control-point values [8,128,5] (pure data movement)."""
    v0 = np.zeros((128, 8, 5), np.float32)
    v1 = np.zeros((128, 8, 5), np.float32)
    for r in range(ROWS):
        for seg in range(NSEG):
            lane = r * NSEG + seg
            v0[lane % 128, lane // 128] = cl_rows[r, seg]
            v1[lane % 128, lane // 128] = cl_rows[r, seg + 1]
    return v0, v1


def build_program():
    from concourse import bass, mybir
    from concourse.tile import TileContext
    fp32 = mybir.dt.float32
    Alu = mybir.AluOpType
    Act = mybir.ActivationFunctionType

    nc = bass.Bass("TRN2", target_bir_lowering=False, debug=False)

    x_in = nc.dram_tensor("x", [ROWS, T], fp32, kind="ExternalInput").ap()
    v0_in = nc.dram_tensor("v0", [128, 8, 5], fp32, kind="ExternalInput").ap()
    v1_in = nc.dram_tensor("v1", [128, 8, 5], fp32, kind="ExternalInput").ap()
    w0_in = nc.dram_tensor("w0", [128, 8, 1], fp32, kind="ExternalInput").ap()
    iota_in = nc.dram_tensor("iota", [128, WIN], fp32, kind="ExternalInput").ap()
    y_out = nc.dram_tensor("y", [ROWS, T], fp32, kind="ExternalOutput").ap()

    st_nas = [nc.dram_tensor(f"st_na{r}", [2, T + 2 * PAD], fp32).ap()
              for r in range(ROWS)]
    st_y = nc.dram_tensor("st_y", [ROWS, T + 2 * PAD], fp32).ap()
    st_cmp = nc.dram_tensor("st_cmp", [128, 6], fp32).ap()
    st_sin = nc.dram_tensor("st_sin", [ROWS, NSTR, 2], fp32).ap()

    import bass_rust

    # Round-robin DMA issuance across otherwise-idle engine queues: the sim
    # trace showed SP at 69% busy issuing every DMA serially while TensorE
    # sat idle. Spreading descriptor generation parallelizes both the issue
    # cost and the hardware DMA queues used.
    _dma_engines = [nc.sync, nc.scalar, nc.gpsimd]
    _dma_rr = [0]

    def dma(out, in_):
        eng = _dma_engines[_dma_rr[0] % len(_dma_engines)]
        _dma_rr[0] += 1
        return eng.dma_start(out=out, in_=in_)

    def win_src(st, r, start, n):
        v = st[r, :].copy()
        v.ap = bass_rust.VecI64Pair([[1028, n], [1, WIN]])
        v.offset = v.offset + start
        return v

    def na_dst(r, start, n, ln):
        # [n segs (stride 1028), 2 planes, ln cols] view into row-r na stage
        v = st_nas[r][0, :].copy()
        v.ap = bass_rust.VecI64Pair([[1028, n], [T + 2 * PAD, 2], [1, ln]])
        v.offset = v.offset + start
        return v

    def scatter_na(ti, tap):
        """Scatter BOTH na planes of the paired tile [128, 2*WIN] at once."""
        tv = tap.rearrange("p (c w) -> p c w", c=2)
        for (tj, part, r, k, sp0, n) in [u for u in RUNS if u[0] == ti]:
            if sp0 == 0:
                base_t = SEGLEN * k
                dma(out=na_dst(r, PAD + base_t, 1, 1029),
                    in_=tv[part:part + 1, :, 2:WIN - 1])
                if n > 1:
                    dma(out=na_dst(r, PAD + SEGLEN * k + 1029, n - 1, 1028),
                        in_=tv[part + 1: part + n, :, 3:WIN - 1])
            else:
                base_t = SEGLEN * k + 1028 * sp0 + 1
                dma(out=na_dst(r, PAD + base_t, n, 1028),
                    in_=tv[part: part + n, :, 3:WIN - 1])
        for r in range(ROWS):
            lane = r * NSEG + NSEG - 1
            tj2, part2 = divmod(lane, 128)
            if tj2 == ti:
                dma(out=na_dst(r, PAD + T - 1, 1, 1),
                    in_=tv[part2:part2 + 1, :, WIN - 1:WIN])

    def scatter_tile(ti, tap, st, off):
        """Write true-segment cols of window tile `ti` to time-linear layout."""
        for (tj, part, r, k, sp0, n) in [u for u in RUNS if u[0] == ti]:
            if sp0 == 0:
                base_t = SEGLEN * k
                dma(out=st[r:r + 1, off + base_t: off + base_t + 1029],
                                  in_=tap[part:part + 1, 2:WIN - 1])
                if n > 1:
                    base_t = SEGLEN * k + 1029
                    dma(
                        out=st[r, off + base_t: off + base_t + (n - 1) * 1028]
                            .rearrange("(a b) -> a b", b=1028),
                        in_=tap[part + 1: part + n, 3:WIN - 1])
            else:
                base_t = SEGLEN * k + 1028 * sp0 + 1
                dma(
                    out=st[r, off + base_t: off + base_t + n * 1028]
                        .rearrange("(a b) -> a b", b=1028),
                    in_=tap[part: part + n, 3:WIN - 1])
        # last sample t = T-1 comes from the final segment's window col 1031
        for r in range(ROWS):
            lane = r * NSEG + NSEG - 1
            tj2, part2 = divmod(lane, 128)
            if tj2 == ti:
                dma(out=st[r:r + 1, off + T - 1: off + T],
                                  in_=tap[part2:part2 + 1, WIN - 1:WIN])

    # affine map composition: prefix along last axis of [P, nblk, L] comps
    def prefix_chain(comps, tmps, P, nblk, L, gp_tmps=None):
        """In-place inclusive prefix of 2x2 affine maps along the last axis.
        Row-1 (m11/m12/q1) runs on DVE, row-2 (m21/m22/q2) on GPSIMD -- the
        two update groups are independent within a step, halving the chain's
        wall time."""
        (m11, m12, m21, m22, q1, q2) = comps
        (t1, t2, t3, t4, t5, t6) = tmps
        if gp_tmps is None:
            gp_tmps = tmps  # caller guarantees disjoint use if shared
        (g1, g2, g3, g4, g5, g6) = gp_tmps
        for k in range(1, L):
            cur = lambda a: a[:, :, k]
            prv = lambda a: a[:, :, k - 1]
            # row 1 on DVE
            nc.vector.tensor_tensor(t1, cur(m11), prv(m11), Alu.mult)
            nc.vector.tensor_tensor(t2, cur(m11), prv(m12), Alu.mult)
            nc.vector.tensor_tensor(t3, cur(m11), prv(q1), Alu.mult)
            nc.vector.tensor_tensor(t4, cur(m12), prv(m21), Alu.mult)
            nc.vector.tensor_tensor(t5, cur(m12), prv(m22), Alu.mult)
            nc.vector.tensor_tensor(t6, cur(m12), prv(q2), Alu.mult)
            nc.vector.tensor_tensor(cur(m11), t1, t4, Alu.add)
            nc.vector.tensor_tensor(cur(m12), t2, t5, Alu.add)
            nc.vector.tensor_tensor(t3, t3, t6, Alu.add)
            nc.vector.tensor_tensor(cur(q1), t3, cur(q1), Alu.add)
            # row 2 on GPSIMD (reads only prv() slices and its own cur() row)
            nc.gpsimd.tensor_tensor(g1, cur(m21), prv(m11), Alu.mult)
            nc.gpsimd.tensor_tensor(g2, cur(m21), prv(m12), Alu.mult)
            nc.gpsimd.tensor_tensor(g3, cur(m21), prv(q1), Alu.mult)
            nc.gpsimd.tensor_tensor(g4, cur(m22), prv(m21), Alu.mult)
            nc.gpsimd.tensor_tensor(g5, cur(m22), prv(m22), Alu.mult)
            nc.gpsimd.tensor_tensor(g6, cur(m22), prv(q2), Alu.mult)
            nc.gpsimd.tensor_tensor(cur(m21), g1, g4, Alu.add)
            nc.gpsimd.tensor_tensor(cur(m22), g2, g5, Alu.add)
            nc.gpsimd.tensor_tensor(g3, g3, g6, Alu.add)
            nc.gpsimd.tensor_tensor(cur(q2), g3, cur(q2), Alu.add)

    with TileContext(nc) as tc:
      with tc.tile_pool(name="xa", bufs=1) as xa_pool:
        t_x = xa_pool.tile([128, CPS, L1], fp32, name="xs")
        t_a1 = xa_pool.tile([128, CPS, L1], fp32, name="a1s")
        t_a2 = xa_pool.tile([128, CPS, L1], fp32, name="a2s")
        dma(out=t_x[:].rearrange("p a b -> p (a b)"),
            in_=x_in.rearrange("r (p s) -> (r p) s", p=NSTR))
        # ---------------- phase A: coefficient generation ----------------
        with tc.tile_pool(name="segc", bufs=1) as sc_pool, \
             tc.tile_pool(name="segp", bufs=2) as sp_pool:
            t_iota = sc_pool.tile([128, WIN], fp32, name="iota_t")
            dma(out=t_iota[:], in_=iota_in)
            # merged control-point loads: [128, 8 tiles, ch] in one DMA each
            t_v0a = sc_pool.tile([128, 8, 5], fp32, name="v0all")
            t_v1a = sc_pool.tile([128, 8, 5], fp32, name="v1all")
            t_w0a = sc_pool.tile([128, 8, 1], fp32, name="w0all")
            t_da = sc_pool.tile([128, 8, 5], fp32, name="dall")
            dma(out=t_v0a[:], in_=v0_in)
            dma(out=t_v1a[:], in_=v1_in)
            dma(out=t_w0a[:], in_=w0_in)
            nc.vector.tensor_tensor(t_da[:], t_v1a[:], t_v0a[:], Alu.subtract)
            for ti in range(8):
                t_v0 = t_v0a[:, ti, :]
                t_d = t_da[:, ti, :]
                t_w0 = t_w0a[:, ti, :]

                t_w = sp_pool.tile([128, WIN], fp32, name=f"w_{ti}", tag="w")
                nc.vector.tensor_scalar(t_w[:], t_iota[:], DELTA, t_w0,
                                        Alu.mult, Alu.add)
                t_t1 = sp_pool.tile([128, WIN], fp32, name=f"t1_{ti}", tag="t1")
                t_t2 = sp_pool.tile([128, WIN], fp32, name=f"t2_{ti}", tag="t2")
                nc.scalar.activation(t_t1[:], t_w[:], Act.Tanh,
                                     bias=t_v0[:, 0:1], scale=t_d[:, 0:1])
                nc.scalar.activation(t_t2[:], t_w[:], Act.Tanh,
                                     bias=t_v0[:, 1:2], scale=t_d[:, 1:2])
                t_na = sp_pool.tile([128, 2 * WIN], fp32, name=f"na_{ti}", tag="na")
                t_na1 = t_na[:, 0:WIN]
                nc.vector.tensor_scalar_mul(t_na1, t_t1[:], -2.0 * STAB)
                t_st2 = sp_pool.tile([128, WIN], fp32, name=f"st2_{ti}", tag="st2")
                nc.vector.tensor_scalar_mul(t_st2[:], t_t2[:], STAB)
                t_vv = sp_pool.tile([128, WIN], fp32, name=f"vv_{ti}", tag="vv")
                nc.vector.tensor_scalar(t_vv[:], t_st2[:], -1.0, 1.0,
                                        Alu.mult, Alu.add)
                t_u = sp_pool.tile([128, WIN], fp32, name=f"u_{ti}", tag="u")
                nc.scalar.activation(t_u[:], t_t1[:], Act.Abs)
                t_uv = sp_pool.tile([128, WIN], fp32, name=f"uv_{ti}", tag="uv")
                nc.gpsimd.tensor_tensor(t_uv[:], t_u[:], t_vv[:], Alu.mult)
                t_na2 = t_na[:, WIN:2 * WIN]
                nc.vector.scalar_tensor_tensor(t_na2, t_uv[:], -STAB, t_st2[:],
                                               Alu.mult, Alu.subtract)
                scatter_na(ti, t_na[:])

        # per-row coefficient loads: RAW-dep only on that row's scatters, so
        # they pipeline under the remaining phase-A tiles
        for r in range(ROWS):
            dma(out=t_a1[r * NSTR:(r + 1) * NSTR].rearrange("p a b -> p (a b)"),
                in_=st_nas[r][0, PAD:PAD + T].rearrange("(p s) -> p s", p=NSTR))
            dma(out=t_a2[r * NSTR:(r + 1) * NSTR].rearrange("p a b -> p (a b)"),
                in_=st_nas[r][1, PAD:PAD + T].rearrange("(p s) -> p s", p=NSTR))

        # ---------------- phases B+C: scan, combine, correction ----------------
        with tc.tile_pool(name="scanp", bufs=1) as pool:
            t_yz = pool.tile([128, CPS, L1], fp32, name="yzs")
            t_h1 = pool.tile([128, CPS, L1 + 2], fp32, name="h1s")
            t_h2 = pool.tile([128, CPS, L1 + 2], fp32, name="h2s")
            nc.vector.memset(t_h1[:, :, 0], 0.0)
            nc.vector.memset(t_h1[:, :, 1], 1.0)
            nc.gpsimd.memset(t_h2[:, :, 0], 1.0)
            nc.gpsimd.memset(t_h2[:, :, 1], 0.0)

            # ladder / state arrays, allocated up front and REUSED as scan
            # scratch before their real use (the scan precedes the ladder)
            BL, LB = 16, 16
            lad = [pool.tile([128, BL, LB], fp32, name=f"lad{i}") for i in range(6)]
            t_al = pool.tile([128, BL, LB], fp32, name="alph")
            t_be = pool.tile([128, BL, LB], fp32, name="beta")
            t_m1 = lad[0][:].rearrange("p a b -> p (a b)")
            t_m2 = lad[1][:].rearrange("p a b -> p (a b)")
            t_g1 = lad[2][:].rearrange("p a b -> p (a b)")
            t_g2 = lad[3][:].rearrange("p a b -> p (a b)")
            t_p1 = t_al[:].rearrange("p a b -> p (a b)")
            t_p2 = t_be[:].rearrange("p a b -> p (a b)")

            for s in range(L1):
                a1s = t_a1[:, :, s]
                a2s = t_a2[:, :, s]
                # y_zero on DVE
                if s == 0:
                    nc.vector.tensor_copy(t_yz[:, :, 0], t_x[:, :, 0])
                elif s == 1:
                    nc.vector.tensor_tensor(t_m1, a1s, t_yz[:, :, 0], Alu.mult)
                    nc.vector.tensor_tensor(t_yz[:, :, 1], t_x[:, :, 1], t_m1,
                                            Alu.add)
                else:
                    nc.vector.tensor_tensor(t_m1, a1s, t_yz[:, :, s - 1], Alu.mult)
                    nc.vector.tensor_tensor(t_m2, a2s, t_yz[:, :, s - 2], Alu.mult)
                    nc.vector.tensor_tensor(t_m1, t_m1, t_m2, Alu.add)
                    nc.vector.tensor_tensor(t_yz[:, :, s], t_x[:, :, s], t_m1,
                                            Alu.add)
                # h1 on DVE
                nc.vector.tensor_tensor(t_g1, a1s, t_h1[:, :, s + 1], Alu.mult)
                nc.vector.tensor_tensor(t_g2, a2s, t_h1[:, :, s], Alu.mult)
                nc.vector.tensor_tensor(t_h1[:, :, s + 2], t_g1, t_g2, Alu.add)
                # h2 on GPSIMD
                nc.gpsimd.tensor_tensor(t_p1, a1s, t_h2[:, :, s + 1], Alu.mult)
                nc.gpsimd.tensor_tensor(t_p2, a2s, t_h2[:, :, s], Alu.mult)
                nc.gpsimd.tensor_tensor(t_h2[:, :, s + 2], t_p1, t_p2, Alu.add)

            # ---- phase C: hierarchical combine ----
            # comps order: m11, m12, m21, m22, q1, q2
            srcs = [t_h1[:, :, L1 + 1], t_h2[:, :, L1 + 1],
                    t_h1[:, :, L1], t_h2[:, :, L1],
                    t_yz[:, :, L1 - 1], t_yz[:, :, L1 - 2]]
            for i in range(6):
                nc.vector.tensor_copy(lad[i][:].rearrange("p a b -> p (a b)"),
                                      srcs[i])
            # scratch: carve ladder temps out of t_al / t_be (still unused here)
            tmps = [t_p1[:, 0:BL], t_p1[:, BL:2 * BL], t_p1[:, 2 * BL:3 * BL],
                    t_p2[:, 0:BL], t_p2[:, BL:2 * BL], t_p2[:, 2 * BL:3 * BL]]
            gtmps = [t_p1[:, 4 * BL:5 * BL], t_p1[:, 5 * BL:6 * BL],
                     t_p1[:, 6 * BL:7 * BL], t_p2[:, 4 * BL:5 * BL],
                     t_p2[:, 5 * BL:6 * BL], t_p2[:, 6 * BL:7 * BL]]
            prefix_chain([a[:] for a in lad], tmps, 128, BL, LB, gp_tmps=gtmps)

            blk = [pool.tile([128, 1, BL], fp32, name=f"blk{i}") for i in range(6)]
            for i in range(6):
                nc.vector.tensor_copy(blk[i][:, 0, :], lad[i][:, :, LB - 1])
            btmp = [t_p1[:, 3 * BL + i:3 * BL + i + 1] for i in range(6)]
            gbtmp = [t_p2[:, 3 * BL + i:3 * BL + i + 1] for i in range(6)]
            prefix_chain([a[:] for a in blk], btmp, 128, 1, BL, gp_tmps=gbtmp)

            # stretch composites -> DRAM (st_cmp[p, c] = comp c of partition p)
            t_cmp = pool.tile([128, 6], fp32, name="cmp")
            for i in range(6):
                nc.vector.tensor_copy(t_cmp[:, i:i + 1], blk[i][:, 0, BL - 1:BL])
            dma(out=st_cmp, in_=t_cmp[:])

            # row-level chain on [4, 32] (4 blocks of 8)
            t_row = pool.tile([4, NSTR, 6], fp32, name="rowc")
            dma(out=t_row[:],
                              in_=st_cmp.rearrange("(r j) c -> r j c", r=ROWS))
            rcomp = [t_row[:, :, i].rearrange("r (b l) -> r b l", b=4)
                     for i in range(6)]
            rtmp = [pool.tile([4, 4], fp32, name=f"rtmp{i}") for i in range(6)]
            rgtmp = [pool.tile([4, 4], fp32, name=f"rgtmp{i}") for i in range(6)]
            prefix_chain(rcomp, [a[:] for a in rtmp], 4, 4, 8,
                         gp_tmps=[a[:] for a in rgtmp])
            rblk = [pool.tile([4, 1, 4], fp32, name=f"rblk{i}") for i in range(6)]
            for i in range(6):
                nc.vector.tensor_copy(rblk[i][:, 0, :], rcomp[i][:, :, 7])
            rbt = [pool.tile([4, 1], fp32, name=f"rbt{i}") for i in range(6)]
            rgbt = [pool.tile([4, 1], fp32, name=f"rgbt{i}") for i in range(6)]
            prefix_chain([a[:] for a in rblk], [a[:] for a in rbt], 4, 1, 4,
                         gp_tmps=[a[:] for a in rgbt])

            # stretch entry states s_str[r, j] (exclusive; global init = 0)
            # qb(b-1): [4, 4] shifted block-inclusive q
            qb1 = pool.tile([4, 4], fp32, name="qb1")
            qb2 = pool.tile([4, 4], fp32, name="qb2")
            nc.vector.memset(qb1[:, 0:1], 0.0)
            nc.vector.memset(qb2[:, 0:1], 0.0)
            nc.vector.tensor_copy(qb1[:, 1:4], rblk[4][:, 0, 0:3])
            nc.vector.tensor_copy(qb2[:, 1:4], rblk[5][:, 0, 0:3])
            # s_incl[r, b, i] = m11*qb1 + m12*qb2 + q1  (for comp 1; same for 2)
            sincl1 = pool.tile([4, 4, 8], fp32, name="sincl1")
            sincl2 = pool.tile([4, 4, 8], fp32, name="sincl2")
            tq1 = pool.tile([4, 4, 8], fp32, name="tq1")
            qb1b = qb1[:].unsqueeze(-1).broadcast_to([4, 4, 8])
            qb2b = qb2[:].unsqueeze(-1).broadcast_to([4, 4, 8])
            nc.vector.tensor_tensor(sincl1[:], rcomp[0], qb1b, Alu.mult)
            nc.vector.tensor_tensor(tq1[:], rcomp[1], qb2b, Alu.mult)
            nc.vector.tensor_tensor(sincl1[:], sincl1[:], tq1[:], Alu.add)
            nc.vector.tensor_tensor(sincl1[:], sincl1[:], rcomp[4], Alu.add)
            nc.vector.tensor_tensor(sincl2[:], rcomp[2], qb1b, Alu.mult)
            nc.vector.tensor_tensor(tq1[:], rcomp[3], qb2b, Alu.mult)
            nc.vector.tensor_tensor(sincl2[:], sincl2[:], tq1[:], Alu.add)
            nc.vector.tensor_tensor(sincl2[:], sincl2[:], rcomp[5], Alu.add)
            # s_entry[j] = s_incl[j-1], s_entry[0] = 0
            sent = pool.tile([4, NSTR, 2], fp32, name="sent")
            nc.vector.memset(sent[:, 0, :], 0.0)
            si1 = sincl1[:].rearrange("r b l -> r (b l)")
            si2 = sincl2[:].rearrange("r b l -> r (b l)")
            nc.vector.tensor_copy(sent[:, 1:NSTR, 0], si1[:, 0:NSTR - 1])
            nc.vector.tensor_copy(sent[:, 1:NSTR, 1], si2[:, 0:NSTR - 1])
            dma(out=st_sin, in_=sent[:])

            # back to scan layout: per-partition stretch entry [128, 2]
            t_sstr = pool.tile([128, 2], fp32, name="sstr")
            dma(out=t_sstr[:],
                              in_=st_sin.rearrange("r j c -> (r j) c"))

            # block entry states within stretch: s_blk [128, 16] (2 comps)
            sb1 = pool.tile([128, BL], fp32, name="sb1")
            sb2 = pool.tile([128, BL], fp32, name="sb2")
            s1 = t_sstr[:, 0:1]
            s2 = t_sstr[:, 1:2]
            # b = 0
            nc.vector.tensor_copy(sb1[:, 0:1], t_sstr[:, 0:1])
            nc.vector.tensor_copy(sb2[:, 0:1], t_sstr[:, 1:2])
            # b >= 1: R_blkpref(b-1) @ s_str + q_blkpref(b-1)
            tb = pool.tile([128, BL - 1], fp32, name="tb")
            nc.vector.tensor_scalar_mul(tb[:], blk[0][:, 0, 0:BL - 1], s1)
            nc.vector.scalar_tensor_tensor(sb1[:, 1:BL], blk[1][:, 0, 0:BL - 1], s2,
                                           tb[:], Alu.mult, Alu.add)
            nc.vector.tensor_tensor(sb1[:, 1:BL], sb1[:, 1:BL], blk[4][:, 0, 0:BL - 1],
                                    Alu.add)
            nc.vector.tensor_scalar_mul(tb[:], blk[2][:, 0, 0:BL - 1], s1)
            nc.vector.scalar_tensor_tensor(sb2[:, 1:BL], blk[3][:, 0, 0:BL - 1], s2,
                                           tb[:], Alu.mult, Alu.add)
            nc.vector.tensor_tensor(sb2[:, 1:BL], sb2[:, 1:BL], blk[5][:, 0, 0:BL - 1],
                                    Alu.add)

            # chunk entry states: alpha/beta [128, 16, 16]
            nc.vector.tensor_copy(t_al[:, :, 0], sb1[:])
            nc.vector.tensor_copy(t_be[:, :, 0], sb2[:])
            sb1b = sb1[:].unsqueeze(-1).broadcast_to([128, BL, LB - 1])
            sb2b = sb2[:].unsqueeze(-1).broadcast_to([128, BL, LB - 1])
            tq = t_x[:, 0:BL, 0:LB - 1]  # x is dead after the scan; reuse as scratch
            nc.vector.tensor_tensor(t_al[:, :, 1:LB], lad[0][:, :, 0:LB - 1], sb1b,
                                    Alu.mult)
            nc.vector.tensor_tensor(tq[:], lad[1][:, :, 0:LB - 1], sb2b, Alu.mult)
            nc.vector.tensor_tensor(t_al[:, :, 1:LB], t_al[:, :, 1:LB], tq[:], Alu.add)
            nc.vector.tensor_tensor(t_al[:, :, 1:LB], t_al[:, :, 1:LB],
                                    lad[4][:, :, 0:LB - 1], Alu.add)
            nc.vector.tensor_tensor(t_be[:, :, 1:LB], lad[2][:, :, 0:LB - 1], sb1b,
                                    Alu.mult)
            nc.vector.tensor_tensor(tq[:], lad[3][:, :, 0:LB - 1], sb2b, Alu.mult)
            nc.vector.tensor_tensor(t_be[:, :, 1:LB], t_be[:, :, 1:LB], tq[:], Alu.add)
            nc.vector.tensor_tensor(t_be[:, :, 1:LB], t_be[:, :, 1:LB],
                                    lad[5][:, :, 0:LB - 1], Alu.add)

            # ---- correction: y = y_zero + alpha*h1 + beta*h2 ----
            alv = t_al[:].rearrange("p a b -> p (a b)")  # [128, 256]
            bev = t_be[:].rearrange("p a b -> p (a b)")
            alb = alv.unsqueeze(-1).broadcast_to([128, CPS, L1])
            beb = bev.unsqueeze(-1).broadcast_to([128, CPS, L1])
            h1v = t_h1[:, :, 2:L1 + 2]
            h2v = t_h2[:, :, 2:L1 + 2]
            # reuse t_x and t_a1 as large temporaries (dead after the scan).
            # Column-split ~65/35 across DVE/GPSIMD: the two chunk ranges form
            # independent dependency chains, so both engines run all four ops
            # concurrently on their slice.
            C0 = 166
            nc.vector.tensor_tensor(t_x[:, 0:C0], h1v[:, 0:C0], alb[:, 0:C0],
                                    Alu.mult)
            nc.vector.tensor_tensor(t_a1[:, 0:C0], h2v[:, 0:C0], beb[:, 0:C0],
                                    Alu.mult)
            nc.vector.tensor_tensor(t_yz[:, 0:C0], t_yz[:, 0:C0], t_x[:, 0:C0],
                                    Alu.add)
            nc.vector.tensor_tensor(t_yz[:, 0:C0], t_yz[:, 0:C0], t_a1[:, 0:C0],
                                    Alu.add)
            nc.gpsimd.tensor_tensor(t_x[:, C0:CPS], h1v[:, C0:CPS],
                                    alb[:, C0:CPS], Alu.mult)
            nc.gpsimd.tensor_tensor(t_a1[:, C0:CPS], h2v[:, C0:CPS],
                                    beb[:, C0:CPS], Alu.mult)
            nc.gpsimd.tensor_tensor(t_yz[:, C0:CPS], t_yz[:, C0:CPS],
                                    t_x[:, C0:CPS], Alu.add)
            nc.gpsimd.tensor_tensor(t_yz[:, C0:CPS], t_yz[:, C0:CPS],
                                    t_a1[:, C0:CPS], Alu.add)

            # store corrected y to time-linear stage (with zeroed lead pad)
            zpad = pool.tile([ROWS, PAD], fp32, name="zpad")
            nc.vector.memset(zpad[:], 0.0)
            dma(out=st_y[:, 0:PAD], in_=zpad[:])
            for r in range(ROWS):
                dma(
                    out=st_y[r, PAD:PAD + T].rearrange("(p s) -> p s", p=NSTR),
                    in_=t_yz[r * NSTR:(r + 1) * NSTR].rearrange("p a b -> p (a b)"))

        # ---------------- phase D: FIR in segment layout ----------------
        with tc.tile_pool(name="firc", bufs=1) as fc_pool, \
             tc.tile_pool(name="firp", bufs=2) as fp_pool:
            t_iota2 = fc_pool.tile([128, WIN], fp32, name="iota2")
            dma(out=t_iota2[:], in_=iota_in)
            t_fv0a = fc_pool.tile([128, 8, 5], fp32, name="fv0all")
            t_fv1a = fc_pool.tile([128, 8, 5], fp32, name="fv1all")
            t_fw0a = fc_pool.tile([128, 8, 1], fp32, name="fw0all")
            t_fda = fc_pool.tile([128, 8, 5], fp32, name="fdall")
            dma(out=t_fv0a[:], in_=v0_in)
            dma(out=t_fv1a[:], in_=v1_in)
            dma(out=t_fw0a[:], in_=w0_in)
            nc.vector.tensor_tensor(t_fda[:], t_fv1a[:], t_fv0a[:], Alu.subtract)
            for ti in range(8):
                t_v0 = t_fv0a[:, ti, :]
                t_d = t_fda[:, ti, :]
                t_w0 = t_fw0a[:, ti, :]
                t_w = fp_pool.tile([128, WIN], fp32, name=f"fw_{ti}", tag="fw")
                nc.vector.tensor_scalar(t_w[:], t_iota2[:], DELTA, t_w0,
                                        Alu.mult, Alu.add)
                t_yw = fp_pool.tile([128, WIN], fp32, name=f"yw_{ti}", tag="yw")
                for (tj, part, r, k, sp0, n) in [u for u in RUNS if u[0] == ti]:
                    start = PAD + SEGLEN * k + 1028 * sp0 - 2
                    dma(out=t_yw[part:part + n, :],
                                      in_=win_src(st_y, r, start, n))
                t_b = [fp_pool.tile([128, WIN], fp32, name=f"b{j}_{ti}", tag=f"b{j}")
                       for j in range(3)]
                for j in range(3):
                    nc.vector.tensor_scalar(t_b[j][:], t_w[:], t_d[:, 2 + j:3 + j],
                                            t_v0[:, 2 + j:3 + j], Alu.mult, Alu.add)
                t_o = fp_pool.tile([128, WIN], fp32, name=f"o_{ti}", tag="o")
                t_f1 = fp_pool.tile([128, WIN - 2], fp32, name=f"f1_{ti}", tag="f1")
                t_f2 = fp_pool.tile([128, WIN - 2], fp32, name=f"f2_{ti}", tag="f2")
                nc.vector.tensor_tensor(t_o[:, 2:], t_b[0][:, 2:], t_yw[:, 2:],
                                        Alu.mult)
                nc.gpsimd.tensor_tensor(t_f1[:], t_b[1][:, 2:], t_yw[:, 1:WIN - 1],
                                        Alu.mult)
                nc.gpsimd.tensor_tensor(t_f2[:], t_b[2][:, 2:], t_yw[:, 0:WIN - 2],
                                        Alu.mult)
                nc.vector.tensor_tensor(t_o[:, 2:], t_o[:, 2:], t_f1[:], Alu.add)
                nc.vector.tensor_tensor(t_o[:, 2:], t_o[:, 2:], t_f2[:], Alu.add)
                scatter_tile(ti, t_o[:], y_out, 0)

    _fix_multi_waits(nc)
    return nc


_NC_CACHE = None


def kernel(x, coeff_logits):
    """Full inputs -> full output, running the Bass kernel on 8 NeuronCores."""
    global _NC_CACHE
    _patch_tile_drain()
    from concourse.bass_utils import run_bass_kernel_spmd

    x = np.ascontiguousarray(np.asarray(x, dtype=np.float32))
    cl = np.ascontiguousarray(np.asarray(coeff_logits, dtype=np.float32))
    if _NC_CACHE is None:
        _NC_CACHE = build_program()
    nc = _NC_CACHE

    w0, iota = host_tables()
    in_maps = []
    for c in range(NCORES):
        rows = slice(c * ROWS, (c + 1) * ROWS)
        v0, v1 = host_v0v1(cl[rows])
        in_maps.append({
            "x": x[rows].copy(),
            "v0": v0, "v1": v1, "w0": w0, "iota": iota,
        })
    import os, time, jax
    do_time = bool(int(os.environ.get("KERNEL_TIME", "0")))
    global LAST_EXEC_NS
    if do_time:
        # capture the final jitted PJRT callable so we can measure marginal
        # device execution time with repeat calls
        cap = {}
        orig_jit = jax.jit

        def capturing_jit(f, **kw):
            j = orig_jit(f, **kw)

            def wrapper(*a, **k):
                cap['fn'], cap['args'] = j, a
                return j(*a, **k)
            return wrapper

        jax.jit = capturing_jit
        try:
            res = run_bass_kernel_spmd(nc, in_maps, list(range(NCORES)))
        finally:
            jax.jit = orig_jit
        try:
            fn, args = cap['fn'], cap['args']
            jax.block_until_ready(fn(*args))  # warm
            K = 5
            t0 = time.perf_counter()
            for _ in range(K):
                o = fn(*args)
            jax.block_until_ready(o)
            LAST_EXEC_NS = int((time.perf_counter() - t0) / K * 1e9)
        except Exception as e:
            print("timing failed:", e)
            LAST_EXEC_NS = -1
    else:
        res = run_bass_kernel_spmd(nc, in_maps, list(range(NCORES)))
    out = np.empty((B, T), np.float32)
    for c in range(NCORES):
        out[c * ROWS:(c + 1) * ROWS] = res.results[c]["y"]
    return out


LAST_EXEC_NS = None



# revision 7
# speedup vs baseline: 2743.4893x; 2743.4893x over previous
"""Trainium2 Bass kernel for nn_BiquadCoeffFilter_31628139167986.

Reference computation (per batch row, T = 262144 samples):
  logits = linear-interp of 256 control points -> T samples (5 channels)
  a1 = 2*tanh(l0)*stab ; a2 = 0.5*((2-|a1|)*tanh(l1)*stab + |a1|)  (stab = 1-1e-3)
  IIR:  y[t] = x[t] - a1[t]*y[t-1] - a2[t]*y[t-2]
  FIR:  out[t] = b0[t]*y[t] + b1[t]*y[t-1] + b2[t]*y[t-2],  b = logits[..., 2:5]

Sharding: pure data parallel, 4 batch rows per core x 8 NeuronCores (SPMD).

Per-core pipeline (v1):
  A. Coefficient generation in SEGMENT-WINDOW layout (partition = one
     interpolation segment window of 1032 samples; the interpolated logit is
     affine in the in-window position, so the Activation engine computes
     tanh(w*d + v0) with per-partition scale/bias).  na1 = -a1 / na2 = -a2
     scattered to per-row time-linear DRAM stages, reloaded per row into the
     scan tile.
  B. Chunked 3-solution scan in SCAN layout [128 partitions = 8192-sample
     stretches, 256 chunks x 32 steps]: per chunk the zero-state response
     y_zero + homogeneous response h1 on DVE, h2 on GPSIMD.
  C. Kogge-Stone prefix over the 256 per-chunk 2x2 affine transfer maps
     (flat [128,256] comp tiles, row-1 on DVE / row-2 on GPSIMD), a [4,32]
     stretch-level KS via a tiny DRAM hop, then per-chunk entry states
     alpha/beta and the correction y = y_zero + alpha*h1 + beta*h2.
  D. FIR in segment-window layout: y staged to DRAM, windows gathered back,
     b coefficients generated on the Activation engine, output scattered
     straight to DRAM.
"""
import sys
sys.path.insert(0, '/opt/trn_rl_repo')
import numpy as np

B, T = 32, 262144
NSEG = 255
SEGLEN = 87381      # (T-1)/3 ; 3 super-blocks x 85 segments per row
SUP = 85
ROWS = 4
NCORES = 8
L1 = 32             # chunk length
NSTR = 32           # stretches per row
STR = T // NSTR     # 8192
CPS = STR // L1     # 256 chunks per stretch
WIN = 1032
PAD = 4
DELTA = float(NSEG) / float(T - 1)
STAB = 1.0 - 1e-3

_PATCHED = False


def _patch_tile_drain():
    """This toolchain allows a single sem wait per instruction; split the tile
    tail-drain's accumulated waits across chained drain instructions."""
    global _PATCHED
    if _PATCHED:
        return
    from concourse import tile, mybir
    from concourse.vector_clock import ScopedClock

    def _drain_and_barrier_split(self, tick_clock, wait_clock):
        drain_inst = self.nc.sync.drain()
        wait_clock.add_sem_waits(
            drain_inst.ins, ScopedClock({None: tick_clock.global_clock}))
        si = drain_inst.ins.sync_info
        waits = list(si.on_wait or []) if si else []
        if len(waits) > 1:
            si.on_wait = waits[:1]
            for i in range(1, len(waits)):
                d2 = self.nc.sync.drain()
                d2.ins.sync_info = mybir.SyncInfo(on_wait=[waits[i]], on_update=[])
        self.nc.all_engine_barrier()
        assert self.sems is not None
        popped = self.nc._tile_sem_poison_stack.pop()
        assert popped is self._sem_poison
        self.nc.clear_and_free_semaphores(list(self.sems.allocated().values()))
        self.nc.all_engine_barrier()

    tile.TileContext._drain_and_barrier = _drain_and_barrier_split
    _PATCHED = True


def _fix_multi_waits(nc):
    """Hoist extra sem waits onto same-engine nops (1-wait codegen limit)."""
    from concourse import mybir

    def make_nop(engine):
        bi = nc.engines[engine].nop(nofuse=True, hint="wait_split")
        inst = bi.ins
        for f in nc.m.functions:
            for bb in f.blocks:
                il = bb.instructions
                if il and il[-1] is inst:
                    bb.instructions = il[:-1]
                    return inst
        raise RuntimeError("nop not found")

    for f in nc.m.functions:
        for bb in f.blocks:
            il = list(bb.instructions)
            out = []
            changed = False
            for inst in il:
                si = getattr(inst, 'sync_info', None)
                waits = list(si.on_wait or []) if si else []
                if len(waits) > 1 and getattr(inst, 'engine', None) is not None:
                    changed = True
                    extra, keep = waits[:-1], waits[-1:]
                    for w in extra:
                        nop = make_nop(inst.engine)
                        nop.sync_info = mybir.SyncInfo(on_wait=[w], on_update=[])
                        out.append(nop)
                    si.on_wait = keep
                out.append(inst)
            if changed:
                bb.instructions = out
    return nc


def _lane_runs():
    """lane = r*255 + 85*k + sp  (row r, super-block k, segment sp).
    Runs of consecutive sp split at 128-partition tile boundaries.
    Returns (tile, part0, r, k, sp0, n)."""
    runs = []
    for r in range(ROWS):
        for k in range(3):
            base = r * NSEG + SUP * k
            sp = 0
            while sp < SUP:
                lane = base + sp
                tile_i, part = divmod(lane, 128)
                n = min(SUP - sp, 128 - part)
                runs.append((tile_i, part, r, k, sp, n))
                sp += n
    return runs


RUNS = _lane_runs()


def host_tables():
    w0 = np.zeros((128, 8, 1), np.float32)
    for r in range(ROWS):
        for k in range(3):
            for sp in range(SUP):
                lane = r * NSEG + SUP * k + sp
                seg = SUP * k + sp
                wstart = SEGLEN * k + 1028 * sp - 2
                w0[lane % 128, lane // 128, 0] = np.float64(wstart) * DELTA - seg
    iota = np.arange(WIN, dtype=np.float32)[None, :].repeat(128, 0)
    return w0, iota


def host_v0v1(cl_rows):
    """Per-lane control-point values [8,128,5] (pure data movement)."""
    v0 = np.zeros((128, 8, 5), np.float32)
    v1 = np.zeros((128, 8, 5), np.float32)
    for r in range(ROWS):
        for seg in range(NSEG):
            lane = r * NSEG + seg
            v0[lane % 128, lane // 128] = cl_rows[r, seg]
            v1[lane % 128, lane // 128] = cl_rows[r, seg + 1]
    return v0, v1


def build_program():
    from concourse import bass, mybir
    from concourse.tile import TileContext
    import bass_rust
    fp32 = mybir.dt.float32
    Alu = mybir.AluOpType
    Act = mybir.ActivationFunctionType

    nc = bass.Bass("TRN2", target_bir_lowering=False, debug=False)

    x_in = nc.dram_tensor("x", [ROWS, T], fp32, kind="ExternalInput").ap()
    v0_in = nc.dram_tensor("v0", [128, 8, 5], fp32, kind="ExternalInput").ap()
    v1_in = nc.dram_tensor("v1", [128, 8, 5], fp32, kind="ExternalInput").ap()
    w0_in = nc.dram_tensor("w0", [128, 8, 1], fp32, kind="ExternalInput").ap()
    iota_in = nc.dram_tensor("iota", [128, WIN], fp32, kind="ExternalInput").ap()
    y_out = nc.dram_tensor("y", [ROWS, T], fp32, kind="ExternalOutput").ap()

    st_nas = [nc.dram_tensor(f"st_na{r}", [2, T + 2 * PAD], fp32).ap()
              for r in range(ROWS)]
    st_y = nc.dram_tensor("st_y", [ROWS, T + 2 * PAD], fp32).ap()
    st_cmp = nc.dram_tensor("st_cmp", [128, 6], fp32).ap()
    st_sin = nc.dram_tensor("st_sin", [ROWS, NSTR, 2], fp32).ap()

    # DMA issuance spread across the queue-capable engines (SP first: it is
    # nearly idle; PE cannot issue DMAs on this target).
    _dma_engines = [nc.sync, nc.scalar, nc.gpsimd]
    _dma_rr = [0]

    def dma(out, in_):
        eng = _dma_engines[_dma_rr[0] % len(_dma_engines)]
        _dma_rr[0] += 1
        return eng.dma_start(out=out, in_=in_)

    def win_src(st, r, start, n):
        v = st[r, :].copy()
        v.ap = bass_rust.VecI64Pair([[1028, n], [1, WIN]])
        v.offset = v.offset + start
        return v

    def na_dst(r, start, n, ln):
        # [n segs (stride 1028), 2 planes, ln cols] view into row-r na stage
        v = st_nas[r][0, :].copy()
        v.ap = bass_rust.VecI64Pair([[1028, n], [T + 2 * PAD, 2], [1, ln]])
        v.offset = v.offset + start
        return v

    def scatter_na(ti, tap):
        """Scatter BOTH na planes of the paired tile [128, 2*WIN] at once."""
        tv = tap.rearrange("p (c w) -> p c w", c=2)
        for (tj, part, r, k, sp0, n) in [u for u in RUNS if u[0] == ti]:
            if sp0 == 0:
                base_t = SEGLEN * k
                dma(out=na_dst(r, PAD + base_t, 1, 1029),
                    in_=tv[part:part + 1, :, 2:WIN - 1])
                if n > 1:
                    dma(out=na_dst(r, PAD + SEGLEN * k + 1029, n - 1, 1028),
                        in_=tv[part + 1: part + n, :, 3:WIN - 1])
            else:
                base_t = SEGLEN * k + 1028 * sp0 + 1
                dma(out=na_dst(r, PAD + base_t, n, 1028),
                    in_=tv[part: part + n, :, 3:WIN - 1])
        for r in range(ROWS):
            lane = r * NSEG + NSEG - 1
            tj2, part2 = divmod(lane, 128)
            if tj2 == ti:
                dma(out=na_dst(r, PAD + T - 1, 1, 1),
                    in_=tv[part2:part2 + 1, :, WIN - 1:WIN])

    def scatter_tile(ti, tap, st, off):
        """Write true-segment cols of window tile `ti` to time-linear layout."""
        for (tj, part, r, k, sp0, n) in [u for u in RUNS if u[0] == ti]:
            if sp0 == 0:
                base_t = SEGLEN * k
                dma(out=st[r:r + 1, off + base_t: off + base_t + 1029],
                    in_=tap[part:part + 1, 2:WIN - 1])
                if n > 1:
                    base_t = SEGLEN * k + 1029
                    dma(
                        out=st[r, off + base_t: off + base_t + (n - 1) * 1028]
                            .rearrange("(a b) -> a b", b=1028),
                        in_=tap[part + 1: part + n, 3:WIN - 1])
            else:
                base_t = SEGLEN * k + 1028 * sp0 + 1
                dma(
                    out=st[r, off + base_t: off + base_t + n * 1028]
                        .rearrange("(a b) -> a b", b=1028),
                    in_=tap[part: part + n, 3:WIN - 1])
        # last sample t = T-1 comes from the final segment's window col 1031
        for r in range(ROWS):
            lane = r * NSEG + NSEG - 1
            tj2, part2 = divmod(lane, 128)
            if tj2 == ti:
                dma(out=st[r:r + 1, off + T - 1: off + T],
                    in_=tap[part2:part2 + 1, WIN - 1:WIN])

    with TileContext(nc) as tc:
      with tc.tile_pool(name="outer", bufs=1) as outer, \
           tc.tile_pool(name="consts", bufs=1) as consts:
        t_x = outer.tile([128, CPS, L1], fp32, name="xs")
        t_a12 = outer.tile([128, 2, CPS, L1], fp32, name="a12")
        t_yz = outer.tile([128, CPS, L1], fp32, name="yzs")
        dma(out=t_x[:].rearrange("p a b -> p (a b)"),
            in_=x_in.rearrange("r (p s) -> (r p) s", p=NSTR))

        t_iota = consts.tile([128, WIN], fp32, name="iota_t")
        t_v0a = consts.tile([128, 8, 5], fp32, name="v0all")
        t_v1a = consts.tile([128, 8, 5], fp32, name="v1all")
        t_w0a = consts.tile([128, 8, 1], fp32, name="w0all")
        t_da = consts.tile([128, 8, 5], fp32, name="dall")
        dma(out=t_iota[:], in_=iota_in)
        dma(out=t_v0a[:], in_=v0_in)
        dma(out=t_v1a[:], in_=v1_in)
        dma(out=t_w0a[:], in_=w0_in)
        nc.vector.tensor_tensor(t_da[:], t_v1a[:], t_v0a[:], Alu.subtract)

        # ---------------- phase A: coefficient generation ----------------
        with tc.tile_pool(name="segp", bufs=2) as sp_pool:
            for ti in range(8):
                t_v0 = t_v0a[:, ti, :]
                t_d = t_da[:, ti, :]
                t_w0 = t_w0a[:, ti, :]

                t_w = sp_pool.tile([128, WIN], fp32, name=f"w_{ti}", tag="w")
                nc.scalar.activation(t_w[:], t_iota[:], Act.Identity,
                                     bias=t_w0[:, 0:1], scale=DELTA)
                t_t1 = sp_pool.tile([128, WIN], fp32, name=f"t1_{ti}", tag="t1")
                t_t2 = sp_pool.tile([128, WIN], fp32, name=f"t2_{ti}", tag="t2")
                nc.scalar.activation(t_t1[:], t_w[:], Act.Tanh,
                                     bias=t_v0[:, 0:1], scale=t_d[:, 0:1])
                nc.scalar.activation(t_t2[:], t_w[:], Act.Tanh,
                                     bias=t_v0[:, 1:2], scale=t_d[:, 1:2])
                t_u = sp_pool.tile([128, WIN], fp32, name=f"u_{ti}", tag="u")
                nc.scalar.activation(t_u[:], t_t1[:], Act.Abs)
                t_na = sp_pool.tile([128, 2 * WIN], fp32, name=f"na_{ti}",
                                    tag="na")
                t_na1 = t_na[:, 0:WIN]
                t_na2 = t_na[:, WIN:2 * WIN]
                nc.vector.tensor_scalar_mul(t_na1, t_t1[:], -2.0 * STAB)
                t_st2 = sp_pool.tile([128, WIN], fp32, name=f"st2_{ti}",
                                     tag="st2")
                nc.vector.tensor_scalar_mul(t_st2[:], t_t2[:], STAB)
                t_vv = sp_pool.tile([128, WIN], fp32, name=f"vv_{ti}", tag="vv")
                nc.vector.tensor_scalar(t_vv[:], t_st2[:], -1.0, 1.0,
                                        Alu.mult, Alu.add)
                t_uv = sp_pool.tile([128, WIN], fp32, name=f"uv_{ti}", tag="uv")
                nc.gpsimd.tensor_tensor(t_uv[:], t_u[:], t_vv[:], Alu.mult)
                nc.vector.scalar_tensor_tensor(t_na2, t_uv[:], -STAB, t_st2[:],
                                               Alu.mult, Alu.subtract)
                scatter_na(ti, t_na[:])

        # per-row coefficient loads: RAW-dep only on that row's scatters, so
        # they pipeline under the remaining phase-A tiles
        a1 = t_a12[:, 0, :, :]
        a2 = t_a12[:, 1, :, :]
        for r in range(ROWS):
            dma(out=t_a12[r * NSTR:(r + 1) * NSTR, 0]
                .rearrange("p a b -> p (a b)"),
                in_=st_nas[r][0, PAD:PAD + T].rearrange("(p s) -> p s", p=NSTR))
            dma(out=t_a12[r * NSTR:(r + 1) * NSTR, 1]
                .rearrange("p a b -> p (a b)"),
                in_=st_nas[r][1, PAD:PAD + T].rearrange("(p s) -> p s", p=NSTR))

        # ---------------- phase B: chunked 3-solution scan ----------------
        with tc.tile_pool(name="scanp", bufs=1) as pool:
            t_h1 = pool.tile([128, CPS, L1], fp32, name="h1s")
            t_h2 = pool.tile([128, CPS, L1], fp32, name="h2s")
            t_tmp1 = pool.tile([128, CPS], fp32, name="sc_t1")
            t_tmp2 = pool.tile([128, CPS], fp32, name="sc_t2")
            t_g1 = pool.tile([128, CPS], fp32, name="gp_t1")
            t_g2 = pool.tile([128, CPS], fp32, name="gp_t2")

            # s = 0:  yz0 = x0 ; h1 col0 = a1_0 ; h2 col0 = a2_0
            nc.scalar.activation(t_yz[:, :, 0], t_x[:, :, 0], Act.Copy)
            nc.scalar.activation(t_h1[:, :, 0], a1[:, :, 0], Act.Copy)
            nc.scalar.activation(t_h2[:, :, 0], a2[:, :, 0], Act.Copy)
            # s = 1
            nc.vector.tensor_tensor(t_tmp1[:], a1[:, :, 1], t_yz[:, :, 0],
                                    Alu.mult)
            nc.vector.tensor_tensor(t_yz[:, :, 1], t_x[:, :, 1], t_tmp1[:],
                                    Alu.add)
            nc.vector.tensor_tensor(t_tmp2[:], a1[:, :, 1], t_h1[:, :, 0],
                                    Alu.mult)
            nc.vector.tensor_tensor(t_h1[:, :, 1], t_tmp2[:], a2[:, :, 1],
                                    Alu.add)
            nc.gpsimd.tensor_tensor(t_h2[:, :, 1], a1[:, :, 1], t_h2[:, :, 0],
                                    Alu.mult)
            # s = 2 .. L1-1
            for s in range(2, L1):
                a1s = a1[:, :, s]
                a2s = a2[:, :, s]
                nc.vector.tensor_tensor(t_tmp1[:], a1s, t_yz[:, :, s - 1],
                                        Alu.mult)
                nc.vector.tensor_tensor(t_tmp2[:], a2s, t_yz[:, :, s - 2],
                                        Alu.mult)
                nc.vector.tensor_tensor(t_tmp1[:], t_tmp1[:], t_tmp2[:],
                                        Alu.add)
                nc.vector.tensor_tensor(t_yz[:, :, s], t_x[:, :, s], t_tmp1[:],
                                        Alu.add)
                nc.vector.tensor_tensor(t_tmp1[:], a1s, t_h1[:, :, s - 1],
                                        Alu.mult)
                nc.vector.tensor_tensor(t_tmp2[:], a2s, t_h1[:, :, s - 2],
                                        Alu.mult)
                nc.vector.tensor_tensor(t_h1[:, :, s], t_tmp1[:], t_tmp2[:],
                                        Alu.add)
                nc.gpsimd.tensor_tensor(t_g1[:], a1s, t_h2[:, :, s - 1],
                                        Alu.mult)
                nc.gpsimd.tensor_tensor(t_g2[:], a2s, t_h2[:, :, s - 2],
                                        Alu.mult)
                nc.gpsimd.tensor_tensor(t_h2[:, :, s], t_g1[:], t_g2[:],
                                        Alu.add)

            # ---------------- phase C: Kogge-Stone combine ----------------
            # comps order: m11, m12, m21, m22, q1, q2
            # carve KS ping-pong buffers + alpha/beta out of t_x (dead now)
            xf = t_x[:].rearrange("p a b -> p (a b)")
            ksA = [xf[:, i * CPS:(i + 1) * CPS] for i in range(6)]
            ksB = [xf[:, (6 + i) * CPS:(7 + i) * CPS] for i in range(6)]
            t_al = xf[:, 12 * CPS:13 * CPS]
            t_be = xf[:, 13 * CPS:14 * CPS]
            srcs = [t_h1[:, :, L1 - 1], t_h2[:, :, L1 - 1],
                    t_h1[:, :, L1 - 2], t_h2[:, :, L1 - 2],
                    t_yz[:, :, L1 - 1], t_yz[:, :, L1 - 2]]
            for i in range(6):
                nc.scalar.activation(ksA[i], srcs[i], Act.Copy)

            def ks_compose(dst, hi, lo, off, w, tmps):
                """dst[c] = hi[c] o lo[c-off] for the w composable entries;
                row1 (m11,m12,q1) on DVE, row2 on GPSIMD."""
                (d11, d12, d21, d22, dq1, dq2) = [d[:, off:off + w] for d in dst]
                (h11, h12, h21, h22, hq1, hq2) = [h[:, off:off + w] for h in hi]
                (l11, l12, l21, l22, lq1, lq2) = [l[:, 0:w] for l in lo]
                (tA, tB, tC, tD) = tmps
                # row 1 (DVE)
                nc.vector.tensor_tensor(tA, h11, l11, Alu.mult)
                nc.vector.tensor_tensor(tB, h12, l21, Alu.mult)
                nc.vector.tensor_tensor(d11, tA, tB, Alu.add)
                nc.vector.tensor_tensor(tA, h11, l12, Alu.mult)
                nc.vector.tensor_tensor(tB, h12, l22, Alu.mult)
                nc.vector.tensor_tensor(d12, tA, tB, Alu.add)
                nc.vector.tensor_tensor(tA, h11, lq1, Alu.mult)
                nc.vector.tensor_tensor(tB, h12, lq2, Alu.mult)
                nc.vector.tensor_tensor(tA, tA, tB, Alu.add)
                nc.vector.tensor_tensor(dq1, tA, hq1, Alu.add)
                # row 2 (GPSIMD)
                nc.gpsimd.tensor_tensor(tC, h21, l11, Alu.mult)
                nc.gpsimd.tensor_tensor(tD, h22, l21, Alu.mult)
                nc.gpsimd.tensor_tensor(d21, tC, tD, Alu.add)
                nc.gpsimd.tensor_tensor(tC, h21, l12, Alu.mult)
                nc.gpsimd.tensor_tensor(tD, h22, l22, Alu.mult)
                nc.gpsimd.tensor_tensor(d22, tC, tD, Alu.add)
                nc.gpsimd.tensor_tensor(tC, h21, lq1, Alu.mult)
                nc.gpsimd.tensor_tensor(tD, h22, lq2, Alu.mult)
                nc.gpsimd.tensor_tensor(tC, tC, tD, Alu.add)
                nc.gpsimd.tensor_tensor(dq2, tC, hq2, Alu.add)

            cur, nxt = ksA, ksB
            off = 1
            while off < CPS:
                for i in range(6):  # pass-through prefix entries
                    nc.scalar.activation(nxt[i][:, 0:off], cur[i][:, 0:off],
                                         Act.Copy)
                w = CPS - off
                ks_compose(nxt, cur, cur, off, w,
                           (t_tmp1[:, 0:w], t_tmp2[:, 0:w],
                            t_g1[:, 0:w], t_g2[:, 0:w]))
                cur, nxt = nxt, cur
                off *= 2

            # stretch composites -> DRAM (st_cmp[p, c] = comp c of partition p)
            t_cmp = pool.tile([128, 6], fp32, name="cmp")
            for i in range(6):
                nc.scalar.activation(t_cmp[:, i:i + 1], cur[i][:, CPS - 1:CPS],
                                     Act.Copy)
            dma(out=st_cmp, in_=t_cmp[:])

            # row-level KS over the 32 stretches of each row (4 partitions)
            t_row = pool.tile([4, NSTR, 6], fp32, name="rowc")
            dma(out=t_row[:], in_=st_cmp.rearrange("(r j) c -> r j c", r=ROWS))
            rA = [pool.tile([4, NSTR], fp32, name=f"rA{i}") for i in range(6)]
            rB = [pool.tile([4, NSTR], fp32, name=f"rB{i}") for i in range(6)]
            rt = [pool.tile([4, NSTR], fp32, name=f"rt{i}") for i in range(4)]
            for i in range(6):
                nc.scalar.activation(rA[i][:], t_row[:, :, i], Act.Copy)
            rcur, rnxt = rA, rB
            off = 1
            while off < NSTR:
                for i in range(6):
                    nc.scalar.activation(rnxt[i][:, 0:off], rcur[i][:, 0:off],
                                         Act.Copy)
                w = NSTR - off
                ks_compose([rr[:] for rr in rnxt], [rr[:] for rr in rcur],
                           [rr[:] for rr in rcur], off, w,
                           tuple(rr[:, 0:w] for rr in (rt[0][:], rt[1][:],
                                                       rt[2][:], rt[3][:])))
                rcur, rnxt = rnxt, rcur
                off *= 2

            # stretch entry states: s_entry[r, j] = prefix[j-1] applied to 0
            # (global init state is zero) => s_entry[j] = (q1, q2)[j-1]
            sent = pool.tile([4, NSTR, 2], fp32, name="sent")
            nc.vector.memset(sent[:, 0, :], 0.0)
            nc.vector.tensor_copy(sent[:, 1:NSTR, 0], rcur[4][:, 0:NSTR - 1])
            nc.vector.tensor_copy(sent[:, 1:NSTR, 1], rcur[5][:, 0:NSTR - 1])
            dma(out=st_sin, in_=sent[:])

            # back to scan layout: per-partition stretch entry [128, 2]
            t_sstr = pool.tile([128, 2], fp32, name="sstr")
            dma(out=t_sstr[:], in_=st_sin.rearrange("r j c -> (r j) c"))

            # chunk entry states: alpha/beta [128, CPS]
            # alpha[0] = s1 ; alpha[c] = m11p[c-1] s1 + m12p[c-1] s2 + q1p[c-1]
            s1 = t_sstr[:, 0:1]
            s2 = t_sstr[:, 1:2]
            nc.vector.tensor_copy(t_al[:, 0:1], s1)
            nc.vector.tensor_copy(t_be[:, 0:1], s2)
            W = CPS - 1
            # TensorScalarPtr (AP-scalar) ops are DVE-only; plain adds on GP
            nc.vector.tensor_scalar_mul(t_tmp1[:, 0:W], cur[0][:, 0:W], s1)
            nc.vector.scalar_tensor_tensor(t_al[:, 1:CPS], cur[1][:, 0:W], s2,
                                           t_tmp1[:, 0:W], Alu.mult, Alu.add)
            nc.gpsimd.tensor_tensor(t_al[:, 1:CPS], t_al[:, 1:CPS],
                                    cur[4][:, 0:W], Alu.add)
            nc.vector.tensor_scalar_mul(t_tmp2[:, 0:W], cur[2][:, 0:W], s1)
            nc.vector.scalar_tensor_tensor(t_be[:, 1:CPS], cur[3][:, 0:W], s2,
                                           t_tmp2[:, 0:W], Alu.mult, Alu.add)
            nc.gpsimd.tensor_tensor(t_be[:, 1:CPS], t_be[:, 1:CPS],
                                    cur[5][:, 0:W], Alu.add)

            # ---- correction: y = y_zero + alpha*h1 + beta*h2 ----
            alb = t_al.unsqueeze(-1).broadcast_to([128, CPS, L1])
            beb = t_be.unsqueeze(-1).broadcast_to([128, CPS, L1])
            # temps: reuse the two a12 planes (dead after the scan)
            tt1 = t_a12[:, 0, :, :]
            tt2 = t_a12[:, 1, :, :]
            C0 = 186  # DVE/GPSIMD column split (~73/27)
            nc.vector.tensor_tensor(tt1[:, 0:C0], t_h1[:, 0:C0], alb[:, 0:C0],
                                    Alu.mult)
            nc.vector.tensor_tensor(tt2[:, 0:C0], t_h2[:, 0:C0], beb[:, 0:C0],
                                    Alu.mult)
            nc.vector.tensor_tensor(t_yz[:, 0:C0], t_yz[:, 0:C0], tt1[:, 0:C0],
                                    Alu.add)
            nc.vector.tensor_tensor(t_yz[:, 0:C0], t_yz[:, 0:C0], tt2[:, 0:C0],
                                    Alu.add)
            nc.gpsimd.tensor_tensor(tt1[:, C0:CPS], t_h1[:, C0:CPS],
                                    alb[:, C0:CPS], Alu.mult)
            nc.gpsimd.tensor_tensor(tt2[:, C0:CPS], t_h2[:, C0:CPS],
                                    beb[:, C0:CPS], Alu.mult)
            nc.gpsimd.tensor_tensor(t_yz[:, C0:CPS], t_yz[:, C0:CPS],
                                    tt1[:, C0:CPS], Alu.add)
            nc.gpsimd.tensor_tensor(t_yz[:, C0:CPS], t_yz[:, C0:CPS],
                                    tt2[:, C0:CPS], Alu.add)

            # store corrected y to time-linear stage (with zeroed lead pad)
            zpad = pool.tile([ROWS, PAD], fp32, name="zpad")
            nc.vector.memset(zpad[:], 0.0)
            dma(out=st_y[:, 0:PAD], in_=zpad[:])
            for r in range(ROWS):
                dma(out=st_y[r, PAD:PAD + T].rearrange("(p s) -> p s", p=NSTR),
                    in_=t_yz[r * NSTR:(r + 1) * NSTR]
                    .rearrange("p a b -> p (a b)"))

        # ------------- phase D: FIR in segment-window layout -------------
        with tc.tile_pool(name="firp", bufs=2) as fp_pool:
            for ti in range(8):
                t_v0 = t_v0a[:, ti, :]
                t_d = t_da[:, ti, :]
                t_w0 = t_w0a[:, ti, :]
                t_w = fp_pool.tile([128, WIN], fp32, name=f"fw_{ti}", tag="fw")
                nc.scalar.activation(t_w[:], t_iota[:], Act.Identity,
                                     bias=t_w0[:, 0:1], scale=DELTA)
                t_yw = fp_pool.tile([128, WIN], fp32, name=f"yw_{ti}", tag="yw")
                for (tj, part, r, k, sp0, n) in [u for u in RUNS if u[0] == ti]:
                    start = PAD + SEGLEN * k + 1028 * sp0 - 2
                    dma(out=t_yw[part:part + n, :],
                        in_=win_src(st_y, r, start, n))
                t_b = [fp_pool.tile([128, WIN], fp32, name=f"b{j}_{ti}",
                                    tag=f"b{j}") for j in range(3)]
                for j in range(3):
                    nc.scalar.activation(t_b[j][:], t_w[:], Act.Identity,
                                         bias=t_v0[:, 2 + j:3 + j],
                                         scale=t_d[:, 2 + j:3 + j])
                t_o = fp_pool.tile([128, WIN], fp32, name=f"o_{ti}", tag="o")
                t_f1 = fp_pool.tile([128, WIN - 2], fp32, name=f"f1_{ti}",
                                    tag="f1")
                t_f2 = fp_pool.tile([128, WIN - 2], fp32, name=f"f2_{ti}",
                                    tag="f2")
                nc.vector.tensor_tensor(t_o[:, 2:], t_b[0][:, 2:], t_yw[:, 2:],
                                        Alu.mult)
                nc.vector.tensor_tensor(t_f1[:], t_b[1][:, 2:],
                                        t_yw[:, 1:WIN - 1], Alu.mult)
                nc.gpsimd.tensor_tensor(t_f2[:], t_b[2][:, 2:],
                                        t_yw[:, 0:WIN - 2], Alu.mult)
                nc.vector.tensor_tensor(t_o[:, 2:], t_o[:, 2:], t_f1[:],
                                        Alu.add)
                nc.vector.tensor_tensor(t_o[:, 2:], t_o[:, 2:], t_f2[:],
                                        Alu.add)
                scatter_tile(ti, t_o[:], y_out, 0)

    _fix_multi_waits(nc)
    return nc


_NC_CACHE = None
LAST_EXEC_NS = None


def _register_ntff_hook():
    """Make antenv.axon_hooks importable and register the ctypes NTFF hook so
    run_bass_kernel_spmd(trace=True) can measure real device exec time."""
    import types
    name = 'antenv.axon_hooks'
    if name not in sys.modules:
        mod = types.ModuleType(name)
        holder = [None]
        mod.set_axon_ntff_profile_hook = lambda h: holder.__setitem__(0, h)
        mod.get_axon_ntff_profile_hook = lambda: holder[0]
        import antenv
        antenv.axon_hooks = mod
        sys.modules[name] = mod
    if sys.modules[name].get_axon_ntff_profile_hook() is None:
        from trn_agent_boot.trn_boot import _ntff_profile_via_ctypes
        hook = _ntff_profile_via_ctypes('/opt/axon/libaxon_pjrt.so')
        sys.modules[name].set_axon_ntff_profile_hook(hook)


def kernel(x, coeff_logits):
    """Full inputs -> full output, running the Bass kernel on 8 NeuronCores."""
    global _NC_CACHE, LAST_EXEC_NS
    _patch_tile_drain()
    from concourse.bass_utils import run_bass_kernel_spmd

    x = np.ascontiguousarray(np.asarray(x, dtype=np.float32))
    cl = np.ascontiguousarray(np.asarray(coeff_logits, dtype=np.float32))
    if _NC_CACHE is None:
        _NC_CACHE = build_program()
    nc = _NC_CACHE

    w0, iota = host_tables()
    in_maps = []
    for c in range(NCORES):
        rows = slice(c * ROWS, (c + 1) * ROWS)
        v0, v1 = host_v0v1(cl[rows])
        in_maps.append({
            "x": x[rows].copy(),
            "v0": v0, "v1": v1, "w0": w0, "iota": iota,
        })
    import os
    mode = os.environ.get("KERNEL_TIME", "0")
    if mode == "ntff":
        # real device timing + perfetto trace via the NTFF profile hook
        _register_ntff_hook()
        from concourse import bass_utils as _bu
        _bu.upload_artifacts = lambda tmpdir: "local://" + tmpdir
        outdir = os.environ.get("PROF_OUT", "/tmp/kernel_prof")
        os.makedirs(outdir, exist_ok=True)
        res = run_bass_kernel_spmd(nc, in_maps, list(range(NCORES)),
                                   tmpdir=outdir, trace=True, trace_cores=[0])
        LAST_EXEC_NS = res.exec_time_ns
    elif mode == "1":
        import time, jax
        cap = {}
        orig_jit = jax.jit

        def capturing_jit(f, **kw):
            j = orig_jit(f, **kw)

            def wrapper(*a, **k):
                cap['fn'], cap['args'] = j, a
                return j(*a, **k)
            return wrapper

        jax.jit = capturing_jit
        try:
            res = run_bass_kernel_spmd(nc, in_maps, list(range(NCORES)))
        finally:
            jax.jit = orig_jit
        try:
            fn, args = cap['fn'], cap['args']
            jax.block_until_ready(fn(*args))  # warm
            K = 5
            t0 = time.perf_counter()
            for _ in range(K):
                o = fn(*args)
            jax.block_until_ready(o)
            LAST_EXEC_NS = int((time.perf_counter() - t0) / K * 1e9)
        except Exception as e:
            print("timing failed:", e)
            LAST_EXEC_NS = -1
    else:
        res = run_bass_kernel_spmd(nc, in_maps, list(range(NCORES)))
    out = np.empty((B, T), np.float32)
    for c in range(NCORES):
        out[c * ROWS:(c + 1) * ROWS] = res.results[c]["y"]
    return out
